# revision 2
# baseline (speedup 1.0000x reference)
"""Trainium2 distributed kernel for the FM/GNN rating model (nn_APM_16733192585590).

Math: rate = x@w_lin + 0.5*sum((xV)^2 - x^2 V^2) + bias_u[uid] + bias_i[iid] + 3
with x = [user_table[uid], word_table[uid], item_table[iid], word_table[iid+NU]].

Decomposition (x = [u | i], V = [V_U ; V_I], G = V_U @ V_I.T, SVD G = A B^T
truncated to rank 126):
  rate_b = P[uid_b] . Q[iid_b]   (a 128-wide dot)
with row P[u] = [U_emb[u]@A, alpha_u[u]+1.5, 1] and
     row Q[i] = [Q_emb[i]@B, 1, alpha_i[i]+1.5]; sigma_126/127 of G ~1e-4 so
rank-126 is exact to ~1e-7. Tables bf16 (rel err ~1e-3 << 2e-2 gate).

Device (v4, per core, batch shard of 2048):
  - 32x indirect_dma_start (INDIRECT1D, resident ucode, int32 offsets, 128
    rows x 256B per instruction) from one combined bf16 table, u/i
    interleaved per tile; raw-block manual semaphores (Tile's auto-sync costs
    ~310ns/instr in sem handling; raw gaps are ~50ns).
  - DVE bf16 mult + reduce per 2-tile chunk, f32 out; out DMA split in two.
Perf notes (this session's traces):
  - gather ucodes cost ~8.6-10ns per INDEX of Q7 software time
    (INDIRECT1D 1103ns/128; DMAGatherAnt ~5.2us/512) regardless of
    instruction batching; the cost-model 0.34ns/desc applies only to regular
    strided SWDGE DMAs. 4096 gathered rows/core -> ~36us Q7 floor.
  - dma_gather needs mlp load_library (~6us IRAM DMA on first call) and
    int16 idxs (32768-row windows); single_packet=True dies >64 descs/engine.
  - fixed overheads: ~7us NEFF preamble, ~2.5us ci DMA, ~5us epilogue
    (sem clears + end barrier).
"""

import numpy as np
import ml_dtypes

from concourse import bacc, bass, mybir
from concourse.bass_utils import run_bass_kernel_spmd

N_USERS = 100000
N_ITEMS = 100000
DIM = 64
EMB = 2 * DIM
R = 128                # 126 sketch dims + 2 alpha sentinels
RANK = 126
BATCH = 16384
N_CORES = 8
SHARD = BATCH // N_CORES      # 2048
P = 128
T = SHARD // P                # 16 tiles -> 32 gathers (u+i)
CHUNK = 2                     # tiles per DVE chunk

_nc_cache = {}


def _build_nc():
    if "nc" in _nc_cache:
        return _nc_cache["nc"]
    f32 = mybir.dt.float32
    bf16 = mybir.dt.bfloat16
    i32 = mybir.dt.int32

    nc = bacc.Bacc(None, target_bir_lowering=False, debug=False)
    cidx = nc.declare_dram_parameter("cidx", [P, 2 * T], i32, isOutput=False)
    ctab = nc.declare_dram_parameter("ctab", [N_USERS + N_ITEMS, R], bf16,
                                     isOutput=False)
    out = nc.declare_dram_parameter("out", [P, T], f32, isOutput=True)

    NG = T // CHUNK  # 8 DVE chunk groups

    from contextlib import ExitStack

    with (
        nc.Block(no_gpsimd_drain=True) as block,
        nc.sbuf_tensor("ci", [P, 2 * T], i32) as ci,
        nc.sbuf_tensor("xa", [P, 2 * T, R], bf16) as xa,
        nc.sbuf_tensor("prod", [P, CHUNK, R], bf16) as prod,
        nc.sbuf_tensor("r", [P, T], f32) as r,
        nc.semaphore("io") as io,
        nc.semaphore("v") as v,
        ExitStack() as stack,
    ):
        # one sem per DVE chunk: each DMA's .then_inc(sem,16) is 16 SDMA
        # engines inc'ing 1 apiece (completion order across DMAs is NOT
        # FIFO), so a shared counter can't prove "first k gathers landed".
        gs = [stack.enter_context(nc.semaphore(f"g{i}")) for i in range(NG)]  # noqa: ANT232

        @block.sync
        def _(sy):
            sy.dma_start(ci[:], cidx[:]).then_inc(io, 16)
            sy.wait_ge(v, NG // 2)
            sy.dma_start(out[:, : T // 2], r[:, : T // 2]).then_inc(io, 16)
            sy.wait_ge(v, NG)
            sy.dma_start(out[:, T // 2 :], r[:, T // 2 :]).then_inc(io, 16)
            sy.wait_ge(io, 48)

        @block.gpsimd
        def _(gp):
            gp.wait_ge(io, 16)
            for k in range(2 * T):
                gp.indirect_dma_start(
                    out=xa[:, k, :],
                    out_offset=None,
                    in_=ctab[:],
                    in_offset=bass.IndirectOffsetOnAxis(
                        ap=ci[:, k : k + 1], axis=0
                    ),
                ).then_inc(gs[k // (2 * CHUNK)], 16)

        @block.vector
        def _(ve):
            for cg in range(NG):
                ve.wait_ge(gs[cg], 16 * 2 * CHUNK)
                c0 = 2 * CHUNK * cg
                ve.tensor_tensor(
                    out=prod[:],
                    in0=xa[:, c0 : c0 + 2 * CHUNK : 2, :],
                    in1=xa[:, c0 + 1 : c0 + 2 * CHUNK : 2, :],
                    op=mybir.AluOpType.mult,
                )
                ve.reduce_sum(
                    r[:, CHUNK * cg : CHUNK * (cg + 1)],
                    prod[:],
                    axis=mybir.AxisListType.X,
                ).then_inc(v, 1)

    nc.finalize()
    _nc_cache["nc"] = nc
    return nc


def _prep_tables(user_table, item_table, word_table, w_lin, V, bias_u, bias_i):
    """Weight-only preprocessing (reusable across batches)."""
    f32 = np.float32
    U_emb = np.concatenate([user_table, word_table[:N_USERS]], axis=1).astype(f32)
    I_emb = np.concatenate(
        [item_table, word_table[N_USERS : N_USERS + N_ITEMS]], axis=1
    ).astype(f32)
    V = np.asarray(V, f32)
    w_lin = np.asarray(w_lin, f32)
    V_U, V_I = V[:EMB], V[EMB:]
    s = (V * V).sum(axis=1)
    ZU = U_emb @ V_U
    alpha_u = (
        np.asarray(bias_u, f32)
        + U_emb @ w_lin[:EMB]
        + 0.5 * (ZU * ZU).sum(axis=1)
        - 0.5 * (U_emb * U_emb) @ s[:EMB]
        + 1.5
    )
    ZI = I_emb @ V_I
    alpha_i = (
        np.asarray(bias_i, f32)
        + I_emb @ w_lin[EMB:]
        + 0.5 * (ZI * ZI).sum(axis=1)
        - 0.5 * (I_emb * I_emb) @ s[EMB:]
        + 1.5
    )
    G = V_U @ V_I.T
    Ug, S, Vt = np.linalg.svd(G)
    A = Ug[:, :RANK] * np.sqrt(S[:RANK])
    B = Vt[:RANK].T * np.sqrt(S[:RANK])
    ctab = np.empty((N_USERS + N_ITEMS, R), f32)
    ctab[:N_USERS, :RANK] = U_emb @ A
    ctab[:N_USERS, RANK] = alpha_u
    ctab[:N_USERS, RANK + 1] = 1.0
    ctab[N_USERS:, :RANK] = I_emb @ B
    ctab[N_USERS:, RANK] = 1.0
    ctab[N_USERS:, RANK + 1] = alpha_i
    return np.ascontiguousarray(ctab.astype(ml_dtypes.bfloat16))


def kernel(
    uid_batch,
    iid_batch,
    n_users,
    user_table,
    item_table,
    word_table,
    w_lin,
    V,
    bias_u,
    bias_i,
    _trace=False,
):
    uid = np.asarray(uid_batch).astype(np.int32)
    iid = np.asarray(iid_batch).astype(np.int32) + N_USERS
    ctab = _prep_tables(
        np.asarray(user_table, np.float32),
        np.asarray(item_table, np.float32),
        np.asarray(word_table, np.float32),
        w_lin,
        V,
        bias_u,
        bias_i,
    )

    nc = _build_nc()
    in_maps = []
    for c in range(N_CORES):
        us = uid[c * SHARD : (c + 1) * SHARD].reshape(P, T)
        is_ = iid[c * SHARD : (c + 1) * SHARD].reshape(P, T)
        cidx = np.empty((P, 2 * T), np.int32)
        cidx[:, 0::2] = us
        cidx[:, 1::2] = is_
        in_maps.append({"cidx": np.ascontiguousarray(cidx), "ctab": ctab})
    res = run_bass_kernel_spmd(
        nc, in_maps, core_ids=list(range(N_CORES)), trace=_trace
    )
    outs = [res.results[c]["out"].reshape(SHARD) for c in range(N_CORES)]
    full = np.concatenate(outs).astype(np.float32)
    if _trace:
        return full, res
    return full


# revision 3
# speedup vs baseline: 1.1250x; 1.1250x over previous
"""Trainium2 distributed kernel for the FM/GNN rating model (nn_APM_16733192585590).

Math: rate = x@w_lin + 0.5*sum((xV)^2 - x^2 V^2) + bias_u[uid] + bias_i[iid] + 3
with x = [user_table[uid], word_table[uid], item_table[iid], word_table[iid+NU]].

Decomposition (x = [u | i], V = [V_U ; V_I], G = V_U @ V_I.T, SVD G = A B^T
truncated to rank 126):
  rate_b = P[uid_b] . Q[iid_b]   (a 128-wide dot)
with row P[u] = [U_emb[u]@A, alpha_u[u]+1.5, 1] and
     row Q[i] = [I_emb[i]@B, 1, alpha_i[i]+1.5]  (alphas fold the row-local
linear + quadratic + bias terms; sigma_126/127 of G are ~1e-4 so rank-126 is
exact to ~1e-7). Tables stored bf16 (rel err ~1e-3 << 2e-2 gate).

Device strategy (v3): 5 dma_gather instructions (DMAGatherAnt, mlp ucode lib)
per core, zero index padding. Measured ucode cost ~8.4ns per index + ~1us
fixed per instruction (the "0.34ns/desc" SWDGE constant applies only to
regular strided DMAs; the gather ucode's per-index software loop is the wall).
dma_gather uses int16 indices => source window <= 32768 rows:
  - host sorts the batch by uid; core c takes 2048 consecutive sorted
    elements, whose uid span is ~12.5k rows -> ONE u-gather (2048 idxs) from
    a per-core 16384-row window of the P table.
  - within the chunk, elements are ranked by iid and split into 4 quartile
    windows of EXACTLY 512 elements; each window's Q-table slice
    [base_w, base_w+32768) is staged per-core so in_ap bases stay static ->
    4 i-gathers (512 idxs each), no dummy slots anywhere.
  - gathered rows land [slot%128, slot//128, 128]; DVE bf16 mult + reduce
    per window -> 2048 slot rates [128,16] f32; host unpermutes.
  - single_packet=False on the 2048-idx gather (HW dies >64 descs/packet).
"""

import numpy as np
import ml_dtypes

from concourse import bacc, bass, mybir
from concourse.bass_utils import run_bass_kernel_spmd
from concourse.library_config import mlp as mlp_lib

N_USERS = 100000
N_ITEMS = 100000
DIM = 64
EMB = 2 * DIM          # 128 combined embedding floats per row
R = 128                # row length: 126 sketch dims + 2 alpha sentinels
RANK = 126
BATCH = 16384
N_CORES = 8
SHARD = BATCH // N_CORES      # 2048
UWIN = 16384                  # per-core u-window rows (chunk span ~12.5k)
IWIN = 32768                  # staged i-window rows (quartile span ~25k)
NWIN = 4
SLOTS_W = SHARD // NWIN       # 512 slots per i-window, exact
SCOL = SHARD // 128           # 16 output columns
WCOL = SLOTS_W // 128         # 4 columns per window

_nc_cache = {}


def _build_nc():
    if "nc" in _nc_cache:
        return _nc_cache["nc"]
    f32 = mybir.dt.float32
    bf16 = mybir.dt.bfloat16
    i16 = mybir.dt.int16

    nc = bacc.Bacc(
        None,
        target_bir_lowering=False,
        debug=False,
        dynamic_dma_scratch_size=32768,
    )
    ucols = SHARD // 16    # 128
    wcols = SLOTS_W // 16  # 32
    cidx = nc.declare_dram_parameter(
        "cidx", [128, ucols + NWIN * wcols], i16, isOutput=False
    )  # u idxs cols [0,128), i idxs window w cols [128+32w, 128+32w+32)
    ctab_u = nc.declare_dram_parameter("ctab_u", [UWIN, R], bf16, isOutput=False)
    ctab_i = nc.declare_dram_parameter(
        "ctab_i", [NWIN * IWIN, R], bf16, isOutput=False
    )
    out = nc.declare_dram_parameter("out", [128, SCOL], f32, isOutput=True)

    with (
        nc.Block(no_gpsimd_drain=True) as block,
        nc.sbuf_tensor("ci", [128, ucols + NWIN * wcols], i16) as ci,
        nc.sbuf_tensor("xu", [128, SCOL, R], bf16) as xu,
        nc.sbuf_tensor("xi", [128, SCOL, R], bf16) as xi,
        nc.sbuf_tensor("prod", [128, WCOL, R], bf16) as prod,
        nc.sbuf_tensor("r", [128, SCOL], f32) as r,
        nc.semaphore("io") as io,
        nc.semaphore("g") as g,
        nc.semaphore("v") as v,
    ):

        @block.sync
        def _(sy):
            sy.dma_start(ci[:], cidx[:]).then_inc(io, 16)
            sy.wait_ge(v, NWIN // 2)
            sy.dma_start(
                out[:, : SCOL // 2], r[:, : SCOL // 2]
            ).then_inc(io, 16)
            sy.wait_ge(v, NWIN)
            sy.dma_start(
                out[:, SCOL // 2 :], r[:, SCOL // 2 :]
            ).then_inc(io, 16)
            sy.wait_ge(io, 48)

        @block.gpsimd
        def _(gp):
            gp.load_library(mlp_lib)
            gp.wait_ge(io, 16)
            gp.dma_gather(
                xu[:], ctab_u[:], ci[:, 0:ucols], SHARD, SHARD, R,
                single_packet=False,
            ).then_inc(g, 16)
            for w in range(NWIN):
                gp.dma_gather(
                    xi[:, w * WCOL : (w + 1) * WCOL, :],
                    ctab_i[w * IWIN : (w + 1) * IWIN, :],
                    ci[:, ucols + w * wcols : ucols + (w + 1) * wcols],
                    SLOTS_W,
                    SLOTS_W,
                    R,
                    single_packet=False,
                ).then_inc(g, 16)

        @block.vector
        def _(ve):
            for w in range(NWIN):
                ve.wait_ge(g, 16 * (w + 2))
                ve.tensor_tensor(
                    out=prod[:],
                    in0=xu[:, w * WCOL : (w + 1) * WCOL, :],
                    in1=xi[:, w * WCOL : (w + 1) * WCOL, :],
                    op=mybir.AluOpType.mult,
                )
                ve.reduce_sum(
                    r[:, w * WCOL : (w + 1) * WCOL],
                    prod[:],
                    axis=mybir.AxisListType.X,
                ).then_inc(v, 1)

    nc.finalize()
    _nc_cache["nc"] = nc
    return nc


def _prep_tables(user_table, item_table, word_table, w_lin, V, bias_u, bias_i):
    """Weight-only preprocessing (reusable across batches)."""
    f32 = np.float32
    U_emb = np.concatenate([user_table, word_table[:N_USERS]], axis=1).astype(f32)
    I_emb = np.concatenate(
        [item_table, word_table[N_USERS : N_USERS + N_ITEMS]], axis=1
    ).astype(f32)
    V = np.asarray(V, f32)
    w_lin = np.asarray(w_lin, f32)
    V_U, V_I = V[:EMB], V[EMB:]
    s = (V * V).sum(axis=1)
    ZU = U_emb @ V_U
    alpha_u = (
        np.asarray(bias_u, f32)
        + U_emb @ w_lin[:EMB]
        + 0.5 * (ZU * ZU).sum(axis=1)
        - 0.5 * (U_emb * U_emb) @ s[:EMB]
        + 1.5
    )
    ZI = I_emb @ V_I
    alpha_i = (
        np.asarray(bias_i, f32)
        + I_emb @ w_lin[EMB:]
        + 0.5 * (ZI * ZI).sum(axis=1)
        - 0.5 * (I_emb * I_emb) @ s[EMB:]
        + 1.5
    )
    G = V_U @ V_I.T
    Ug, S, Vt = np.linalg.svd(G)
    A = Ug[:, :RANK] * np.sqrt(S[:RANK])
    B = Vt[:RANK].T * np.sqrt(S[:RANK])
    bf = ml_dtypes.bfloat16
    urow = np.empty((N_USERS, R), f32)
    urow[:, :RANK] = U_emb @ A
    urow[:, RANK] = alpha_u
    urow[:, RANK + 1] = 1.0
    irow = np.zeros((N_ITEMS + IWIN, R), f32)  # padded for window overhang
    irow[:N_ITEMS, :RANK] = I_emb @ B
    irow[:N_ITEMS, RANK] = 1.0
    irow[:N_ITEMS, RANK + 1] = alpha_i
    return np.ascontiguousarray(urow.astype(bf)), np.ascontiguousarray(
        irow.astype(bf)
    )


def _wrap16(idx_list):
    """[n] int array -> [128, n//16] int16 SBUF image (i at [i%16, i//16],
    replicated across the 8 16-partition core groups)."""
    n = len(idx_list)
    a = np.asarray(idx_list, np.int16).reshape(n // 16, 16).T  # [16, n/16]
    return np.tile(a, (8, 1))


def kernel(
    uid_batch,
    iid_batch,
    n_users,
    user_table,
    item_table,
    word_table,
    w_lin,
    V,
    bias_u,
    bias_i,
    _trace=False,
):
    uid = np.asarray(uid_batch).astype(np.int64)
    iid = np.asarray(iid_batch).astype(np.int64)
    urow, irow = _prep_tables(
        np.asarray(user_table, np.float32),
        np.asarray(item_table, np.float32),
        np.asarray(word_table, np.float32),
        w_lin,
        V,
        bias_u,
        bias_i,
    )

    order = np.argsort(uid, kind="stable")
    in_maps = []
    slot_elem = []  # per core: original batch position per slot
    for c in range(N_CORES):
        ch = order[c * SHARD : (c + 1) * SHARD]
        u_lo = int(uid[ch].min())
        u_lo = min(u_lo, N_USERS - UWIN)
        assert int(uid[ch].max()) - u_lo < UWIN
        sub = np.argsort(iid[ch], kind="stable")
        ch = ch[sub]  # iid-ranked; window w = ranks [512w, 512w+512)
        iw = iid[ch].reshape(NWIN, SLOTS_W)
        bases = iw[:, 0].copy()
        assert (iw[:, -1] - bases < IWIN).all()
        i_idx = (iw - bases[:, None]).astype(np.int16).reshape(-1)
        u_idx = (uid[ch] - u_lo).astype(np.int16)
        ci = np.concatenate(
            [_wrap16(u_idx)]
            + [
                _wrap16(i_idx[w * SLOTS_W : (w + 1) * SLOTS_W])
                for w in range(NWIN)
            ],
            axis=1,
        )
        ctab_i = np.concatenate(
            [irow[b : b + IWIN] for b in bases], axis=0
        )
        in_maps.append(
            {
                "cidx": np.ascontiguousarray(ci),
                "ctab_u": np.ascontiguousarray(urow[u_lo : u_lo + UWIN]),
                "ctab_i": np.ascontiguousarray(ctab_i),
            }
        )
        slot_elem.append(ch)

    nc = _build_nc()
    res = run_bass_kernel_spmd(
        nc, in_maps, core_ids=list(range(N_CORES)), trace=_trace
    )
    full = np.empty(BATCH, np.float32)
    for c in range(N_CORES):
        flat = res.results[c]["out"].T.reshape(-1)  # slot s -> [s%128, s//128]
        full[slot_elem[c]] = flat
    if _trace:
        return full, res
    return full


# revision 4
# speedup vs baseline: 1.2515x; 1.1125x over previous
"""Trainium2 distributed kernel for the FM/GNN rating model (nn_APM_16733192585590).

Math: rate = x@w_lin + 0.5*sum((xV)^2 - x^2 V^2) + bias_u[uid] + bias_i[iid] + 3
with x = [user_table[uid], word_table[uid], item_table[iid], word_table[iid+NU]].

Decomposition (x = [u | i], V = [V_U ; V_I], G = V_U @ V_I.T, SVD G = A B^T
truncated to rank 126):
  rate_b = P[uid_b] . Q[iid_b]   (a 128-wide dot)
with row P[u] = [U_emb[u]@A, alpha_u[u]+1.5, 1] and
     row Q[i] = [I_emb[i]@B, 1, alpha_i[i]+1.5]  (alphas fold the row-local
linear + quadratic + bias terms; sigma_126/127 of G are ~1e-4 so rank-126 is
exact to ~1e-7). Tables stored bf16 (rel err ~1e-3 << 2e-2 gate).

Device strategy (v3): 5 dma_gather instructions (DMAGatherAnt, mlp ucode lib)
per core, zero index padding. Measured ucode cost ~8.4ns per index + ~1us
fixed per instruction (the "0.34ns/desc" SWDGE constant applies only to
regular strided DMAs; the gather ucode's per-index software loop is the wall).
dma_gather uses int16 indices => source window <= 32768 rows:
  - host sorts the batch by uid; core c takes 2048 consecutive sorted
    elements, whose uid span is ~12.5k rows -> ONE u-gather (2048 idxs) from
    a per-core 16384-row window of the P table.
  - within the chunk, elements are ranked by iid and split into 4 quartile
    windows of EXACTLY 512 elements; each window's Q-table slice
    [base_w, base_w+32768) is staged per-core so in_ap bases stay static ->
    4 i-gathers (512 idxs each), no dummy slots anywhere.
  - gathered rows land [slot%128, slot//128, 128]; DVE bf16 mult + reduce
    per window -> 2048 slot rates [128,16] f32; host unpermutes.
  - single_packet=False on the 2048-idx gather (HW dies >64 descs/packet).
"""

import numpy as np
import ml_dtypes

from concourse import bacc, bass, mybir
from concourse.bass_utils import run_bass_kernel_spmd
from concourse.library_config import mlp as mlp_lib

N_USERS = 100000
N_ITEMS = 100000
DIM = 64
EMB = 2 * DIM          # 128 combined embedding floats per row
R = 128                # row length: 126 sketch dims + 2 alpha sentinels
RANK = 126
BATCH = 16384
N_CORES = 8
SHARD = BATCH // N_CORES      # 2048
UWIN = 16384                  # per-core u-window rows (chunk span ~12.5k)
IWIN = 32768                  # staged i-window rows (quartile span ~25k)
NWIN = 4
SLOTS_W = SHARD // NWIN       # 512 slots per i-window, exact
SCOL = SHARD // 128           # 16 output columns
WCOL = SLOTS_W // 128         # 4 columns per window

_nc_cache = {}


def _build_nc():
    if "nc" in _nc_cache:
        return _nc_cache["nc"]
    f32 = mybir.dt.float32
    bf16 = mybir.dt.bfloat16
    i16 = mybir.dt.int16

    nc = bacc.Bacc(
        None,
        target_bir_lowering=False,
        debug=False,
        dynamic_dma_scratch_size=32768,
        num_swdge_queues=2,
    )
    ucols = SHARD // 16    # 128
    wcols = SLOTS_W // 16  # 32
    cidx = nc.declare_dram_parameter(
        "cidx", [128, ucols + NWIN * wcols], i16, isOutput=False
    )  # u idxs cols [0,128), i idxs window w cols [128+32w, 128+32w+32)
    ctab_u = nc.declare_dram_parameter("ctab_u", [UWIN, R], bf16, isOutput=False)
    ctab_i = nc.declare_dram_parameter(
        "ctab_i", [NWIN * IWIN, R], bf16, isOutput=False
    )
    out = nc.declare_dram_parameter("out", [128, SCOL], f32, isOutput=True)

    with (
        nc.Block(no_gpsimd_drain=True) as block,
        nc.sbuf_tensor("ci", [128, ucols + NWIN * wcols], i16) as ci,
        nc.sbuf_tensor("xu", [128, SCOL, R], bf16) as xu,
        nc.sbuf_tensor("xi", [128, SCOL, R], bf16) as xi,
        nc.sbuf_tensor("prod", [128, WCOL, R], bf16) as prod,
        nc.sbuf_tensor("r", [128, SCOL], f32) as r,
        nc.semaphore("io") as io,
        nc.semaphore("g") as g,
        nc.semaphore("gu") as gu,
        nc.semaphore("v") as v,
    ):

        @block.sync
        def _(sy):
            sy.dma_start(ci[:], cidx[:]).then_inc(io, 16)
            sy.wait_ge(v, NWIN // 2)
            sy.dma_start(
                out[:, : SCOL // 2], r[:, : SCOL // 2]
            ).then_inc(io, 16)
            sy.wait_ge(v, NWIN)
            sy.dma_start(
                out[:, SCOL // 2 :], r[:, SCOL // 2 :]
            ).then_inc(io, 16)
            sy.wait_ge(io, 48)

        @block.gpsimd
        def _(gp):
            gp.load_library(mlp_lib)
            gp.wait_ge(io, 16)
            gp.dma_gather(
                xu[:], ctab_u[:], ci[:, 0:ucols], SHARD, SHARD, R,
                single_packet=False,
            ).then_inc(gu, 16)
            for w in range(NWIN):
                gp.dma_gather(
                    xi[:, w * WCOL : (w + 1) * WCOL, :],
                    ctab_i[w * IWIN : (w + 1) * IWIN, :],
                    ci[:, ucols + w * wcols : ucols + (w + 1) * wcols],
                    SLOTS_W,
                    SLOTS_W,
                    R,
                    single_packet=False,
                    queue_num=1,
                ).then_inc(g, 16)

        @block.vector
        def _(ve):
            ve.wait_ge(gu, 16)
            for w in range(NWIN):
                ve.wait_ge(g, 16 * (w + 1))
                ve.tensor_tensor(
                    out=prod[:],
                    in0=xu[:, w * WCOL : (w + 1) * WCOL, :],
                    in1=xi[:, w * WCOL : (w + 1) * WCOL, :],
                    op=mybir.AluOpType.mult,
                )
                ve.reduce_sum(
                    r[:, w * WCOL : (w + 1) * WCOL],
                    prod[:],
                    axis=mybir.AxisListType.X,
                ).then_inc(v, 1)

    nc.finalize()
    _nc_cache["nc"] = nc
    return nc


def _prep_tables(user_table, item_table, word_table, w_lin, V, bias_u, bias_i):
    """Weight-only preprocessing (reusable across batches)."""
    f32 = np.float32
    U_emb = np.concatenate([user_table, word_table[:N_USERS]], axis=1).astype(f32)
    I_emb = np.concatenate(
        [item_table, word_table[N_USERS : N_USERS + N_ITEMS]], axis=1
    ).astype(f32)
    V = np.asarray(V, f32)
    w_lin = np.asarray(w_lin, f32)
    V_U, V_I = V[:EMB], V[EMB:]
    s = (V * V).sum(axis=1)
    ZU = U_emb @ V_U
    alpha_u = (
        np.asarray(bias_u, f32)
        + U_emb @ w_lin[:EMB]
        + 0.5 * (ZU * ZU).sum(axis=1)
        - 0.5 * (U_emb * U_emb) @ s[:EMB]
        + 1.5
    )
    ZI = I_emb @ V_I
    alpha_i = (
        np.asarray(bias_i, f32)
        + I_emb @ w_lin[EMB:]
        + 0.5 * (ZI * ZI).sum(axis=1)
        - 0.5 * (I_emb * I_emb) @ s[EMB:]
        + 1.5
    )
    G = V_U @ V_I.T
    Ug, S, Vt = np.linalg.svd(G)
    A = Ug[:, :RANK] * np.sqrt(S[:RANK])
    B = Vt[:RANK].T * np.sqrt(S[:RANK])
    bf = ml_dtypes.bfloat16
    urow = np.empty((N_USERS, R), f32)
    urow[:, :RANK] = U_emb @ A
    urow[:, RANK] = alpha_u
    urow[:, RANK + 1] = 1.0
    irow = np.zeros((N_ITEMS + IWIN, R), f32)  # padded for window overhang
    irow[:N_ITEMS, :RANK] = I_emb @ B
    irow[:N_ITEMS, RANK] = 1.0
    irow[:N_ITEMS, RANK + 1] = alpha_i
    return np.ascontiguousarray(urow.astype(bf)), np.ascontiguousarray(
        irow.astype(bf)
    )


def _wrap16(idx_list):
    """[n] int array -> [128, n//16] int16 SBUF image (i at [i%16, i//16],
    replicated across the 8 16-partition core groups)."""
    n = len(idx_list)
    a = np.asarray(idx_list, np.int16).reshape(n // 16, 16).T  # [16, n/16]
    return np.tile(a, (8, 1))


def kernel(
    uid_batch,
    iid_batch,
    n_users,
    user_table,
    item_table,
    word_table,
    w_lin,
    V,
    bias_u,
    bias_i,
    _trace=False,
):
    uid = np.asarray(uid_batch).astype(np.int64)
    iid = np.asarray(iid_batch).astype(np.int64)
    urow, irow = _prep_tables(
        np.asarray(user_table, np.float32),
        np.asarray(item_table, np.float32),
        np.asarray(word_table, np.float32),
        w_lin,
        V,
        bias_u,
        bias_i,
    )

    order = np.argsort(uid, kind="stable")
    in_maps = []
    slot_elem = []  # per core: original batch position per slot
    for c in range(N_CORES):
        ch = order[c * SHARD : (c + 1) * SHARD]
        u_lo = int(uid[ch].min())
        u_lo = min(u_lo, N_USERS - UWIN)
        assert int(uid[ch].max()) - u_lo < UWIN
        sub = np.argsort(iid[ch], kind="stable")
        ch = ch[sub]  # iid-ranked; window w = ranks [512w, 512w+512)
        iw = iid[ch].reshape(NWIN, SLOTS_W)
        bases = iw[:, 0].copy()
        assert (iw[:, -1] - bases < IWIN).all()
        i_idx = (iw - bases[:, None]).astype(np.int16).reshape(-1)
        u_idx = (uid[ch] - u_lo).astype(np.int16)
        ci = np.concatenate(
            [_wrap16(u_idx)]
            + [
                _wrap16(i_idx[w * SLOTS_W : (w + 1) * SLOTS_W])
                for w in range(NWIN)
            ],
            axis=1,
        )
        ctab_i = np.concatenate(
            [irow[b : b + IWIN] for b in bases], axis=0
        )
        in_maps.append(
            {
                "cidx": np.ascontiguousarray(ci),
                "ctab_u": np.ascontiguousarray(urow[u_lo : u_lo + UWIN]),
                "ctab_i": np.ascontiguousarray(ctab_i),
            }
        )
        slot_elem.append(ch)

    nc = _build_nc()
    res = run_bass_kernel_spmd(
        nc, in_maps, core_ids=list(range(N_CORES)), trace=_trace
    )
    full = np.empty(BATCH, np.float32)
    for c in range(N_CORES):
        flat = res.results[c]["out"].T.reshape(-1)  # slot s -> [s%128, s//128]
        full[slot_elem[c]] = flat
    if _trace:
        return full, res
    return full
